# revision 1
# baseline (speedup 1.0000x reference)
"""Trainium2 Bass kernel for nn_Cell_61856118996994 (GNN message passing).

Strategy
--------
Row-shard the 50000 nodes across 8 NeuronCores (6250 rows/core).  The 10
spmms of the reference are fused into 4 "passes", one per accumulation
target (states 1..3 and the final output); each pass is a fused edge list
over (adjacency, source-state) terms with per-edge weight*val.

Per pass, each core processes the edges whose *destination* row falls in
its row range:
  - per-edge gather of the 128-wide f32 source row via dma_gather
    (512B descriptors = full DMA-bus efficiency),
  - segment-sum on the TensorEngine: one-hot matrices (built on the DVE
    with a broadcast iota-compare) matmul'd against the gathered rows,
    accumulating 64-row windows in PSUM, flushed into an SBUF accumulator,
  - AllGather of the produced state shard so the next pass can gather it.

The affine (x@W+b) runs as a K-chunked matmul with a rank-1 bias matmul;
LayerNorm + exact-erf GELU run on the final accumulator.

SPMD: one program runs on all 8 cores, so every (gather-bank, window)
group is padded to the max count over the 8 cores (rounded to 32-edge
quanta); padding edges carry slot=-1 (one-hot kills them).
"""
import sys

sys.path.insert(0, "/opt/trn_rl_repo")

import numpy as np

# ---------------------------------------------------------------- constants
N_NODES = 50000
N_ADJ = 6
N_EDGES = 800000
DP = 256          # prev hidden
D = 128           # hidden
NC = 8            # cores
RPC = N_NODES // NC       # 6250 rows per core
R = 64            # PSUM window rows
NW = (RPC + R - 1) // R   # 98 windows
NTILE_ACC = (RPC + 127) // 128   # 49 row-tiles in the accumulator
QUANT = 64        # group padding quantum (edges); PE base partition must be
                  # in {0, 32, 64}, so 64-quanta keep piece bases at {0, 64}
CHUNK = 8192      # edges per superchunk (idx/sv DMA + one-hot granularity)
GCALL = 1024      # max edges per dma_gather call (SWDGE ring limit)
BANKROWS = 32768  # int16 gather index range per bank
CSTR = [0, 2, 4]
CSTRL = [0, 2, 4, 5]
DT_STATE = "f32"  # state dtype


def _build_terms(idxes_seq0, idxes_seq1, idxes_res0, idxes_res1,
                 ws_seq0, ws_seq1, ws_res0, ws_res1):
    """4 passes; each a list of merged (src_state, adj_k, weight)."""
    t = [[] for _ in range(4)]
    t[0] = [(0, int(idxes_seq0[0]), float(ws_seq0[0]))]
    t[1] = [(1, int(idxes_seq0[1]), float(ws_seq0[1])),
            (0, int(idxes_res0[0]), float(ws_res0[0]))]
    t[2] = [(2, int(idxes_seq0[2]), float(ws_seq0[2])),
            (0, int(idxes_res0[1]), float(ws_res0[1])),
            (1, int(idxes_res0[2]), float(ws_res0[2]))]
    t[3] = [(3, CSTR[int(idxes_seq1[0])], float(ws_seq1[0]))]
    t[3] += [(i, CSTRL[int(idxes_res1[i])], float(ws_res1[i])) for i in range(3)]
    merged = []
    for terms in t:
        d = {}
        for s, k, w in terms:
            d[(s, k)] = d.get((s, k), 0.0) + w
        merged.append(sorted((s, k, w) for (s, k), w in d.items()))
    return merged


class PassSched:
    """Static (SPMD-shared) schedule + per-core data arrays for one pass."""
    __slots__ = ("EP", "NT", "banks", "chunks", "groups", "idx16", "sv")


def _build_pass(terms, adj_rows, adj_cols, adj_vals, n_nodes=N_NODES,
                rpc=RPC, r_win=R, quant=QUANT, chunk_edges=CHUNK,
                bankrows=BANKROWS, ncores=NC):
    nw = (rpc + r_win - 1) // r_win
    rows = np.concatenate([adj_rows[k] for (s, k, w) in terms])
    cols = np.concatenate([adj_cols[k] for (s, k, w) in terms])
    vals = np.concatenate([adj_vals[k].astype(np.float64) * w
                           for (s, k, w) in terms]).astype(np.float32)
    srcs = np.concatenate([np.full(len(adj_rows[k]), si, np.int64)
                           for si, (s, k, w) in enumerate(terms)])
    # banks: per source term, split by bankrows
    banks = []   # (src_state, lo_row, hi_row)
    bank_of_src_half = {}
    for si, (s, k, w) in enumerate(terms):
        nb = (n_nodes + bankrows - 1) // bankrows
        for h in range(nb):
            bank_of_src_half[(si, h)] = len(banks)
            banks.append((s, h * bankrows, min(n_nodes, (h + 1) * bankrows)))
    nbank = len(banks)
    nb_per_src = (n_nodes + bankrows - 1) // bankrows
    bank_id = srcs * nb_per_src + (cols // bankrows)
    bidx = (cols % bankrows).astype(np.int64)

    core = rows // rpc
    local = rows - core * rpc
    win = local // r_win
    slot = (local - win * r_win).astype(np.float32)
    key = bank_id * nw + win

    # per-core sorted data + group counts
    per_core = []
    cnts = np.zeros((ncores, nbank * nw), np.int64)
    for c in range(ncores):
        sel = np.flatnonzero(core == c)
        ks = key[sel]
        o = np.argsort(ks, kind="stable")
        sel = sel[o]
        ks = ks[o]
        cnts[c] = np.bincount(ks, minlength=nbank * nw)
        per_core.append((ks, bidx[sel], slot[sel], vals[sel]))

    static = cnts.max(axis=0)
    static = ((static + quant - 1) // quant) * quant   # [nbank*nw]
    static2 = static.reshape(nbank, nw)
    bank_tot = static2.sum(axis=1)
    bank_pad = (-bank_tot) % 128
    # group offsets in the padded stream
    group_off = np.zeros(nbank * nw, np.int64)
    off = 0
    bank_span = []   # (e0, e1) per bank, padded
    for b in range(nbank):
        b0 = off
        for w in range(nw):
            group_off[b * nw + w] = off
            off += static2[b, w]
        off += bank_pad[b]
        bank_span.append((b0, off))
    EP = off
    NT = EP // 128

    # fill per-core padded arrays
    idx16 = np.zeros((ncores, EP), np.int16)
    slot_a = np.full((ncores, EP), -1.0, np.float32)
    val_a = np.zeros((ncores, EP), np.float32)
    for c in range(ncores):
        ks, bx, sl, vl = per_core[c]
        if len(ks) == 0:
            continue
        run_start_pos = np.flatnonzero(np.diff(ks, prepend=-1))
        run_lens = np.diff(np.append(run_start_pos, len(ks)))
        rank = np.arange(len(ks)) - np.repeat(run_start_pos, run_lens)
        dest = group_off[ks] + rank
        idx16[c, dest] = bx.astype(np.int16)
        slot_a[c, dest] = sl
        val_a[c, dest] = vl

    # gather-idx layout: position j -> partition j%16, col j//16, replicated x8
    idxw = np.zeros((ncores, 128, EP // 16), np.int16)
    for c in range(ncores):
        w16 = idx16[c].reshape(EP // 16, 16).T
        for rep in range(8):
            idxw[c, rep * 16:(rep + 1) * 16, :] = w16
    # slot/val layout: [128, 2, NT]; [p, 0, t] = slot of edge t*128+p
    sv = np.zeros((ncores, 128, 2, NT), np.float32)
    for c in range(ncores):
        sv[c, :, 0, :] = slot_a[c].reshape(NT, 128).T
        sv[c, :, 1, :] = val_a[c].reshape(NT, 128).T

    # chunk list: (bank_index, e0, e1)
    chunks = []
    for b in range(nbank):
        e0, e1 = bank_span[b]
        e = e0
        while e < e1:
            ee = min(e + chunk_edges, e1)
            chunks.append((b, e, ee))
            e = ee
    chunk_starts = np.array([c[1] for c in chunks])

    # groups: (win, [(col, p0, p1, chunk_idx)]) in stream order
    groups = []
    for b in range(nbank):
        for w in range(nw):
            g0 = int(group_off[b * nw + w])
            g1 = g0 + int(static2[b, w])
            if g1 == g0:
                continue
            pieces = []
            e = g0
            while e < g1:
                col = e // 128
                p0 = e - col * 128
                p1 = min(g1 - col * 128, 128)
                ck = int(np.searchsorted(chunk_starts, e, side="right") - 1)
                pieces.append((col, p0, p1, ck))
                e = col * 128 + p1
            groups.append((w, pieces))

    ps = PassSched()
    ps.EP, ps.NT, ps.banks, ps.chunks, ps.groups = EP, NT, banks, chunks, groups
    ps.idx16, ps.sv = idxw, sv
    return ps


def _build_program(scheds):
    """Build the SPMD Bass/Tile program. Returns (nc, input name list)."""
    import concourse.bass as bass
    import concourse.tile as tile
    from concourse import bacc, mybir

    dt = mybir.dt.float32
    nc = bacc.Bacc("TRN2", target_bir_lowering=False, debug=False,
                   enable_asserts=False, num_devices=NC)

    xT_d = nc.dram_tensor("xT", [DP, NTILE_ACC * 128], dt,
                          kind="ExternalInput").ap()
    W_d = nc.dram_tensor("W", [DP, D], dt, kind="ExternalInput").ap()
    b_d = nc.dram_tensor("bias", [1, D], dt, kind="ExternalInput").ap()
    ones_d = nc.dram_tensor("ones", [1, 128], dt, kind="ExternalInput").ap()
    iota_d = nc.dram_tensor("iota", [128, R], dt, kind="ExternalInput").ap()
    idx_d, sv_d = [], []
    for p, ps in enumerate(scheds):
        idx_d.append(nc.dram_tensor(f"idx{p}", [128, ps.EP // 16],
                                    mybir.dt.int16, kind="ExternalInput").ap())
        sv_d.append(nc.dram_tensor(f"sv{p}", [128, 2, ps.NT], dt,
                                   kind="ExternalInput").ap())
    out_d = nc.dram_tensor("out", [RPC, D], dt, kind="ExternalOutput").ap()

    stage = globals().get("BUILD_STAGE", 5)
    with tile.TileContext(nc) as tc:
        with (
            tc.tile_pool(name="persist", bufs=1) as pp,
            tc.tile_pool(name="dram", bufs=1, space="DRAM") as dram,
        ):
            iota_s = pp.tile([128, R], dt)
            nc.sync.dma_start(iota_s[:], iota_d[:])
            acc = pp.tile([128, NTILE_ACC, D], dt)
            states = []
            for t in range(4):
                st = dram.tile([N_NODES, D], dt, addr_space="Shared",
                               name=f"state{t}")
                states.append(st)
            bounces = []
            for t in range(4):
                bn = dram.tile([RPC, D], dt, name=f"bounce{t}")
                bounces.append(bn)

            FT = RPC // 128          # full 128-row tiles
            REMR = RPC - FT * 128    # leftover rows

            def acc_to(dst):
                # acc rows r = 128*c + p  ->  dst[r]
                nc.sync.dma_start(
                    dst[:FT * 128].rearrange("(c p) f -> p c f", p=128),
                    acc[:, :FT, :])
                if REMR:
                    nc.sync.dma_start(dst[FT * 128:RPC],
                                      acc[0:REMR, FT, :])

            # ---------------- pass 0: h0 = x @ W + b ----------------
            with (
                tc.tile_pool(name="p0", bufs=1) as p0,
                tc.tile_pool(name="ps0", bufs=4, space="PSUM") as ps0,
            ):
                xT_s = p0.tile([128, 2, NTILE_ACC * 128], dt)
                nc.sync.dma_start(
                    xT_s[:], xT_d.rearrange("(c k) r -> k c r", k=128))
                W_s = p0.tile([128, 2, D], dt)
                nc.sync.dma_start(
                    W_s[:], W_d.rearrange("(c k) n -> k c n", k=128))
                ones_s = p0.tile([1, 128], dt)
                nc.sync.dma_start(ones_s[:], ones_d[:])
                b_s = p0.tile([1, D], dt)
                nc.sync.dma_start(b_s[:], b_d[:])
                for t in range(NTILE_ACC):
                    pw = ps0.tile([128, D], dt)
                    for c in range(2):
                        nc.tensor.matmul(
                            pw[:], xT_s[:, c, t * 128:(t + 1) * 128],
                            W_s[:, c, :], start=(c == 0), stop=False)
                    nc.tensor.matmul(pw[:], ones_s[:], b_s[:],
                                     start=False, stop=True)
                    nc.scalar.copy(acc[:, t, :], pw[:])
            if stage >= 1:
                acc_to(bounces[0])
                nc.gpsimd.collective_compute(
                    "AllGather", bass.mybir.AluOpType.bypass,
                    replica_groups=[list(range(NC))],
                    ins=[bounces[0][:].opt()], outs=[states[0][:].opt()])

            # ---------------- passes 1..4: fused spmm ----------------
            run_passes = scheds if stage >= 4 else (scheds[:1] if stage >= 2 else [])
            for p, ps in enumerate(run_passes):
                with (
                    tc.tile_pool(name=f"g{p}", bufs=2) as gp,
                    tc.tile_pool(name=f"m{p}", bufs=3) as mp,
                    tc.tile_pool(name=f"psum{p}", bufs=6, space="PSUM") as pspool,
                ):
                    nc.vector.memset(acc[:], 0.0)
                    chunk_tiles = {}
                    # group emission bookkeeping: emit a group once its
                    # last piece's chunk has been emitted
                    glast = {}
                    for gi, (w, pieces) in enumerate(ps.groups):
                        glast.setdefault(pieces[-1][3], []).append(gi)
                    for ck, (b, e0, e1) in enumerate(ps.chunks):
                        ne = e1 - e0
                        nt = ne // 128
                        s_state, lo, hi = ps.banks[b]
                        idx_t = mp.tile([128, CHUNK // 16],
                                        bass.mybir.dt.int16, tag="idx")
                        nc.sync.dma_start(idx_t[:, :ne // 16],
                                          idx_d[p][:, e0 // 16:e1 // 16])
                        sv_t = mp.tile([128, 2, CHUNK // 128], dt, tag="sv")
                        nc.sync.dma_start(sv_t[:, :, :nt],
                                          sv_d[p][:, :, e0 // 128:e1 // 128])
                        g_t = gp.tile([128, CHUNK // 128, D], dt, tag="g")
                        # HW limit: <=1024 descriptors per dma_gather call
                        for sub in range(0, ne, GCALL):
                            se = min(sub + GCALL, ne)
                            nc.gpsimd.dma_gather(
                                g_t[:, sub // 128:se // 128, :],
                                states[s_state][lo:hi, :],
                                idx_t[:, sub // 16:se // 16],
                                num_idxs=se - sub,
                                num_idxs_reg=se - sub, elem_size=D)
                        oh_t = gp.tile([128, CHUNK // 128, R], dt, tag="oh")
                        nc.vector.tensor_tensor(
                            oh_t[:, :nt, :],
                            iota_s[:].unsqueeze(1).broadcast_to([128, nt, R]),
                            sv_t[:, 0, :nt].unsqueeze(2).broadcast_to(
                                [128, nt, R]),
                            bass.mybir.AluOpType.is_equal)
                        nc.vector.tensor_tensor(
                            oh_t[:, :nt, :], oh_t[:, :nt, :],
                            sv_t[:, 1, :nt].unsqueeze(2).broadcast_to(
                                [128, nt, R]),
                            bass.mybir.AluOpType.mult)
                        chunk_tiles[ck] = (g_t, oh_t)
                        if stage < 3:
                            continue
                        for gi in glast.get(ck, ()):
                            w, pieces = ps.groups[gi]
                            pw = pspool.tile([R, D], dt, tag="pw")
                            np_ = len(pieces)
                            for pi, (col, p0_, p1_, ck_) in enumerate(pieces):
                                gt, ot = chunk_tiles[ck_]
                                cl = col - ps.chunks[ck_][1] // 128
                                nc.tensor.matmul(
                                    pw[:], ot[p0_:p1_, cl, :],
                                    gt[p0_:p1_, cl, :],
                                    start=(pi == 0), stop=(pi == np_ - 1))
                            po = (w % 2) * R
                            nc.vector.tensor_add(
                                acc[po:po + R, w // 2, :],
                                acc[po:po + R, w // 2, :], pw[:])
                    if p < 3 and stage >= 4:
                        acc_to(bounces[p + 1])
                        nc.gpsimd.collective_compute(
                            "AllGather", bass.mybir.AluOpType.bypass,
                            replica_groups=[list(range(NC))],
                            ins=[bounces[p + 1][:].opt()],
                            outs=[states[p + 1][:].opt()])

            # ---------------- LayerNorm + GELU ----------------
            if stage < 5:
                FT2 = RPC // 128
                nc.sync.dma_start(
                    out_d[:FT2 * 128].rearrange("(c p) f -> p c f", p=128),
                    acc[:, :FT2, :])
                if RPC - FT2 * 128:
                    nc.sync.dma_start(out_d[FT2 * 128:RPC],
                                      acc[0:RPC - FT2 * 128, FT2, :])
                return nc
            with tc.tile_pool(name="ln", bufs=1) as lp:
                NTA = NTILE_ACC
                eps_t = lp.tile([128, 1], dt)
                nc.vector.memset(eps_t[:], 1e-5)
                zero_t = lp.tile([128, 1], dt)
                nc.vector.memset(zero_t[:], 0.0)
                ms = lp.tile([128, NTA, 1], dt)
                nc.vector.reduce_sum(ms[:], acc[:],
                                     axis=bass.mybir.AxisListType.X)
                mu_t = lp.tile([128, NTA, 1], dt)
                nc.scalar.mul(mu_t[:], ms[:], 1.0 / D)
                xm = lp.tile([128, NTA, D], dt)
                nc.vector.tensor_tensor(
                    xm[:], acc[:], mu_t[:].broadcast_to([128, NTA, D]),
                    bass.mybir.AluOpType.subtract)
                sq = lp.tile([128, NTA, D], dt)
                nc.scalar.square(sq[:], xm[:])
                vs = lp.tile([128, NTA, 1], dt)
                nc.vector.reduce_sum(vs[:], sq[:],
                                     axis=bass.mybir.AxisListType.X)
                std = lp.tile([128, NTA, 1], dt)
                nc.scalar.activation(
                    std[:], vs[:], bass.mybir.ActivationFunctionType.Sqrt,
                    bias=eps_t[:], scale=1.0 / D)
                rinv = lp.tile([128, NTA, 1], dt)
                nc.vector.reciprocal(rinv[:], std[:])
                normed = lp.tile([128, NTA, D], dt)
                nc.vector.tensor_tensor(
                    normed[:], xm[:], rinv[:].broadcast_to([128, NTA, D]),
                    bass.mybir.AluOpType.mult)
                gl = lp.tile([128, NTA, D], dt)
                if globals().get("SIM_NO_GELU", False):
                    nc.scalar.copy(gl[:], normed[:])
                else:
                    nc.scalar.activation(
                        gl[:], normed[:],
                        bass.mybir.ActivationFunctionType.Gelu,
                        bias=zero_t[:])
                FT2 = RPC // 128
                nc.sync.dma_start(
                    out_d[:FT2 * 128].rearrange("(c p) f -> p c f", p=128),
                    gl[:, :FT2, :])
                if RPC - FT2 * 128:
                    nc.sync.dma_start(out_d[FT2 * 128:RPC],
                                      gl[0:RPC - FT2 * 128, FT2, :])
    return nc


def _prepare(inputs):
    """Build schedule + program + in_maps. Returns (nc, in_maps)."""
    x = np.asarray(inputs["x"], np.float32)
    adj_rows = np.asarray(inputs["adj_rows"])
    adj_cols = np.asarray(inputs["adj_cols"])
    adj_vals = np.asarray(inputs["adj_vals"], np.float32)
    W = np.asarray(inputs["W"], np.float32)
    b = np.asarray(inputs["b"], np.float32)

    terms = _build_terms(
        np.asarray(inputs["idxes_seq0"]), np.asarray(inputs["idxes_seq1"]),
        np.asarray(inputs["idxes_res0"]), np.asarray(inputs["idxes_res1"]),
        np.asarray(inputs["ws_seq0"]), np.asarray(inputs["ws_seq1"]),
        np.asarray(inputs["ws_res0"]), np.asarray(inputs["ws_res1"]))
    scheds = [_build_pass(t, adj_rows, adj_cols, adj_vals) for t in terms]
    globals()["_last_scheds"] = scheds
    nc = _build_program(scheds)
    nc.compile()
    from concourse.bass_interp import get_hw_module
    nc.m = get_hw_module(nc.m)

    iota = np.broadcast_to(np.arange(R, dtype=np.float32), (128, R)).copy()
    ones = np.ones((1, 128), np.float32)
    b_row = b.reshape(1, D)
    in_maps = []
    for c in range(NC):
        xs = x[c * RPC:(c + 1) * RPC]
        xs = np.concatenate(
            [xs, np.zeros((NTILE_ACC * 128 - RPC, DP), np.float32)], 0)
        m = {"xT": np.ascontiguousarray(xs.T), "W": W, "bias": b_row,
             "ones": ones, "iota": iota}
        for p, ps in enumerate(scheds):
            m[f"idx{p}"] = ps.idx16[c]
            m[f"sv{p}"] = ps.sv[c]
        in_maps.append(m)
    return nc, in_maps


def kernel(**inputs) -> np.ndarray:
    nc, in_maps = _prepare(inputs)
    from concourse import bass2jax
    results = bass2jax.run_bass_via_pjrt(nc, in_maps, n_cores=NC)
    return np.concatenate([results[c]["out"] for c in range(NC)], axis=0)



# revision 9
# speedup vs baseline: 69.7126x; 69.7126x over previous
"""Trainium2 Bass kernel for nn_Cell_61856118996994 (GNN message passing).

Strategy
--------
Row-shard the 50000 nodes across 8 NeuronCores (6250 rows/core).  The
reference's 10 spmms run as 4 "passes", one per accumulation target
(states 1..3 and the final output); each pass is a list of
(source-state, adjacency, weight) terms.

Per term, each core processes the edges whose *destination* row falls in
its row range:
  - per-edge gather of the 128-wide bf16 source row via dma_gather,
  - segment-sum on the TensorEngine: one-hot matrices (built on the DVE
    with a broadcast iota-compare, scaled by weight*val) matmul'd against
    the gathered rows, accumulating 128-row windows in PSUM, flushed into
    an f32 SBUF accumulator,
  - AllGather of the produced state shard (bf16) so later passes can
    gather it.

Edge schedules are built once per *distinct adjacency* and shared by all
terms that use that adjacency (the per-term weight is folded into the
one-hot on device), so the shipped edge data is ~6 streams instead of 10.
Everything shipped over the host link is minimal-width: gather indices as
unreplicated [16, E/16] int16 (replicated to 128 partitions on device),
slots as int8, values and node features as bf16.

SPMD: one program runs on all 8 cores, so every (gather-bank, window)
group is padded to the max count over the 8 cores (rounded to 64-edge
quanta); padding edges carry slot=-1 (one-hot kills them).
"""
import sys

sys.path.insert(0, "/opt/trn_rl_repo")

import numpy as np

# ---------------------------------------------------------------- constants
N_NODES = 50000
N_ADJ = 6
N_EDGES = 800000
DP = 256          # prev hidden
D = 128           # hidden
NC = 8            # cores
RPC = N_NODES // NC       # 6250 rows per core
R = 128           # PSUM window rows (one full PSUM bank)
NW = (RPC + R - 1) // R   # 49 windows
NTILE_ACC = (RPC + 127) // 128   # 49 row-tiles in the accumulator
QUANT = 64        # group padding quantum (edges); PE base partition must be
                  # in {0, 64} for spans up to 64, so 64-quanta keep piece
                  # bases at {0, 64}
CHUNK = 16384     # edges per superchunk (idx/slot/val DMA + one-hot size)
GCALL = 1024      # max edges per dma_gather call (SWDGE ring limit)
BANKROWS = 32768  # int16 gather index range per bank
CSTR = [0, 2, 4]
CSTRL = [0, 2, 4, 5]


def _build_terms(idxes_seq0, idxes_seq1, idxes_res0, idxes_res1,
                 ws_seq0, ws_seq1, ws_res0, ws_res1):
    """4 passes; each a list of merged (src_state, adj_k, weight)."""
    t = [[] for _ in range(4)]
    t[0] = [(0, int(idxes_seq0[0]), float(ws_seq0[0]))]
    t[1] = [(1, int(idxes_seq0[1]), float(ws_seq0[1])),
            (0, int(idxes_res0[0]), float(ws_res0[0]))]
    t[2] = [(2, int(idxes_seq0[2]), float(ws_seq0[2])),
            (0, int(idxes_res0[1]), float(ws_res0[1])),
            (1, int(idxes_res0[2]), float(ws_res0[2]))]
    t[3] = [(3, CSTR[int(idxes_seq1[0])], float(ws_seq1[0]))]
    t[3] += [(i, CSTRL[int(idxes_res1[i])], float(ws_res1[i])) for i in range(3)]
    merged = []
    for terms in t:
        d = {}
        for s, k, w in terms:
            d[(s, k)] = d.get((s, k), 0.0) + w
        merged.append(sorted((s, k, w) for (s, k), w in d.items()))
    return merged


class AdjSched:
    """Static (SPMD-shared) schedule + per-core data for one adjacency."""
    __slots__ = ("EP", "NT", "banks", "chunks", "groups", "idx16", "slot8",
                 "val32")


def _build_adj(rows, cols, vals, n_nodes=N_NODES, rpc=RPC, r_win=R,
               quant=QUANT, chunk_edges=CHUNK, bankrows=BANKROWS, ncores=NC):
    """Schedule for one adjacency: per-core padded edge streams ordered by
    (source-bank, destination-window)."""
    nw = (rpc + r_win - 1) // r_win
    nbank = (n_nodes + bankrows - 1) // bankrows
    banks = [(h * bankrows, min(n_nodes, (h + 1) * bankrows))
             for h in range(nbank)]
    bank_id = (cols // bankrows).astype(np.int64)
    bidx = (cols % bankrows).astype(np.int64)

    core = rows // rpc
    local = rows - core * rpc
    win = local // r_win
    slot = (local - win * r_win).astype(np.int64)
    key = bank_id * nw + win

    cnts = np.zeros((ncores, nbank * nw), np.int64)
    per_core = []
    for c in range(ncores):
        sel = np.flatnonzero(core == c)
        ks = key[sel]
        o = np.argsort(ks, kind="stable")
        sel = sel[o]
        cnts[c] = np.bincount(ks[o], minlength=nbank * nw)
        per_core.append((ks[o], bidx[sel], slot[sel], vals[sel]))

    static = cnts.max(axis=0)
    static = ((static + quant - 1) // quant) * quant   # [nbank*nw]
    static2 = static.reshape(nbank, nw)
    bank_tot = static2.sum(axis=1)
    bank_pad = (-bank_tot) % 128
    group_off = np.zeros(nbank * nw, np.int64)
    off = 0
    bank_span = []   # (e0, e1) per bank, padded
    for b in range(nbank):
        b0 = off
        for w in range(nw):
            group_off[b * nw + w] = off
            off += static2[b, w]
        off += bank_pad[b]
        bank_span.append((b0, off))
    EP = off
    NT = EP // 128

    idx16 = np.zeros((ncores, EP), np.int16)
    slot_a = np.full((ncores, EP), -1, np.int8)
    val_a = np.zeros((ncores, EP), np.float32)
    for c in range(ncores):
        ks, bx, sl, vl = per_core[c]
        if len(ks) == 0:
            continue
        run_start_pos = np.flatnonzero(np.diff(ks, prepend=-1))
        run_lens = np.diff(np.append(run_start_pos, len(ks)))
        rank = np.arange(len(ks)) - np.repeat(run_start_pos, run_lens)
        dest = group_off[ks] + rank
        idx16[c, dest] = bx.astype(np.int16)
        slot_a[c, dest] = sl.astype(np.int8)
        val_a[c, dest] = vl

    # gather-idx layout: position j -> partition j%16, col j//16 (the on-
    # device loader replicates these 16 partitions across all 128)
    idxw = np.ascontiguousarray(
        idx16.reshape(ncores, EP // 16, 16).transpose(0, 2, 1))
    # slot/val layout: [128, NT]; [p, t] = field of edge t*128+p
    slot8 = np.ascontiguousarray(
        slot_a.reshape(ncores, NT, 128).transpose(0, 2, 1))
    val32 = np.ascontiguousarray(
        val_a.reshape(ncores, NT, 128).transpose(0, 2, 1))

    chunks = []
    for b in range(nbank):
        e0, e1 = bank_span[b]
        e = e0
        while e < e1:
            ee = min(e + chunk_edges, e1)
            chunks.append((b, e, ee))
            e = ee
    chunk_starts = np.array([c[1] for c in chunks])

    # groups: (win, [(col, p0, p1, chunk_idx)]) in stream order
    groups = []
    for b in range(nbank):
        for w in range(nw):
            g0 = int(group_off[b * nw + w])
            g1 = g0 + int(static2[b, w])
            if g1 == g0:
                continue
            pieces = []
            e = g0
            while e < g1:
                col = e // 128
                p0 = e - col * 128
                p1 = min(g1 - col * 128, 128)
                ck = int(np.searchsorted(chunk_starts, e, side="right") - 1)
                pieces.append((col, p0, p1, ck))
                e = col * 128 + p1
            groups.append((w, pieces))

    ps = AdjSched()
    ps.EP, ps.NT, ps.banks, ps.chunks, ps.groups = EP, NT, banks, chunks, groups
    ps.idx16, ps.slot8, ps.val32 = idxw, slot8, val32
    return ps


def _build_program(scheds, terms):
    """Build the SPMD Bass/Tile program.

    scheds: {adj_k: AdjSched} for each distinct adjacency used.
    terms: 4 lists of (src_state, adj_k, weight).
    """
    import concourse.bass as bass
    import concourse.tile as tile
    from concourse import bacc, mybir

    f32 = mybir.dt.float32
    bf16 = mybir.dt.bfloat16
    i8 = mybir.dt.int8
    nc = bacc.Bacc("TRN2", target_bir_lowering=False, debug=False,
                   enable_asserts=False, num_devices=NC)

    xT_d = nc.dram_tensor("xT", [DP, NTILE_ACC * 128], bf16,
                          kind="ExternalInput").ap()
    W_d = nc.dram_tensor("W", [DP, D], bf16, kind="ExternalInput").ap()
    b_d = nc.dram_tensor("bias", [1, D], bf16, kind="ExternalInput").ap()
    ones_d = nc.dram_tensor("ones", [1, 128], bf16, kind="ExternalInput").ap()
    iota_d = nc.dram_tensor("iota", [128, R], i8, kind="ExternalInput").ap()
    adj_keys = sorted(scheds)
    idx_d, slot_d, val_d = {}, {}, {}
    for k in adj_keys:
        ps = scheds[k]
        idx_d[k] = nc.dram_tensor(f"idx{k}", [16, ps.EP // 16],
                                  mybir.dt.int16, kind="ExternalInput").ap()
        slot_d[k] = nc.dram_tensor(f"slot{k}", [128, ps.NT], i8,
                                   kind="ExternalInput").ap()
        val_d[k] = nc.dram_tensor(f"val{k}", [128, ps.NT], bf16,
                                  kind="ExternalInput").ap()
    out_d = nc.dram_tensor("out", [RPC, D], bf16, kind="ExternalOutput").ap()

    with tile.TileContext(nc) as tc:
        with (
            tc.tile_pool(name="persist", bufs=1) as pp,
            tc.tile_pool(name="dram", bufs=1, space="DRAM") as dram,
        ):
            iota_s = pp.tile([128, R], i8)
            nc.sync.dma_start(iota_s[:], iota_d[:])
            acc = pp.tile([128, NTILE_ACC, D], f32)
            acc16 = pp.tile([128, NTILE_ACC, D], bf16)
            states = []
            for t in range(4):
                st = dram.tile([N_NODES, D], bf16, addr_space="Shared",
                               name=f"state{t}")
                states.append(st)
            bounces = []
            for t in range(4):
                bn = dram.tile([RPC, D], bf16, name=f"bounce{t}")
                bounces.append(bn)

            FT = RPC // 128          # full 128-row tiles
            REMR = RPC - FT * 128    # leftover rows

            def acc16_to(dst):
                # acc16 rows r = 128*c + p  ->  dst[r]
                nc.sync.dma_start(
                    dst[:FT * 128].rearrange("(c p) f -> p c f", p=128),
                    acc16[:, :FT, :])
                if REMR:
                    nc.sync.dma_start(dst[FT * 128:RPC],
                                      acc16[0:REMR, FT, :])

            def publish(p):
                acc16_to(bounces[p])
                nc.gpsimd.collective_compute(
                    "AllGather", bass.mybir.AluOpType.bypass,
                    replica_groups=[list(range(NC))],
                    ins=[bounces[p][:].opt()], outs=[states[p][:].opt()])

            def dump_acc16():
                FT2 = RPC // 128
                nc.sync.dma_start(
                    out_d[:FT2 * 128].rearrange("(c p) f -> p c f", p=128),
                    acc16[:, :FT2, :])
                if RPC - FT2 * 128:
                    nc.sync.dma_start(out_d[FT2 * 128:RPC],
                                      acc16[0:RPC - FT2 * 128, FT2, :])

            dbg = globals().get("DEBUG_STAGE", None)

            # ---------------- pass 0: h0 = x @ W + b ----------------
            with (
                tc.tile_pool(name="p0", bufs=1) as p0,
                tc.tile_pool(name="ps0", bufs=2, space="PSUM") as ps0,
            ):
                xT_s = p0.tile([128, 2, NTILE_ACC * 128], bf16)
                nc.sync.dma_start(
                    xT_s[:], xT_d.rearrange("(c k) r -> k c r", k=128))
                W_s = p0.tile([128, 2, D], bf16)
                nc.sync.dma_start(
                    W_s[:], W_d.rearrange("(c k) n -> k c n", k=128))
                ones_s = p0.tile([1, 128], bf16)
                nc.sync.dma_start(ones_s[:], ones_d[:])
                b_s = p0.tile([1, D], bf16)
                nc.sync.dma_start(b_s[:], b_d[:])
                for t in range(NTILE_ACC):
                    pw = ps0.tile([128, D], f32)
                    for c in range(2):
                        nc.tensor.matmul(
                            pw[:], xT_s[:, c, t * 128:(t + 1) * 128],
                            W_s[:, c, :], start=(c == 0), stop=False)
                    nc.tensor.matmul(pw[:], ones_s[:], b_s[:],
                                     start=False, stop=True)
                    nc.scalar.copy(acc16[:, t, :], pw[:])
            if dbg == 0:
                dump_acc16()
                return nc
            publish(0)

            # ------------- spmm passes: terms[q] -> states[q+1] / final -------------
            for q in range(4 if dbg is None else dbg):
                with (
                    tc.tile_pool(name=f"g{q}", bufs=2) as gp,
                    tc.tile_pool(name=f"m{q}", bufs=3) as mp,
                    tc.tile_pool(name=f"psum{q}", bufs=4, space="PSUM") as pspool,
                ):
                    nc.vector.memset(acc[:], 0.0)
                    for s_state, k, wgt in terms[q]:
                        ps = scheds[k]
                        glast = {}
                        for gi, (w, pieces) in enumerate(ps.groups):
                            glast.setdefault(pieces[-1][3], []).append(gi)
                        chunk_tiles = {}
                        for ck, (b, e0, e1) in enumerate(ps.chunks):
                            ne = e1 - e0
                            nt = ne // 128
                            lo, hi = ps.banks[b]
                            idx_t = mp.tile([128, CHUNK // 16],
                                            bass.mybir.dt.int16, tag="idx")
                            for rep in range(8):
                                nc.sync.dma_start(
                                    idx_t[rep * 16:(rep + 1) * 16, :ne // 16],
                                    idx_d[k][:, e0 // 16:e1 // 16])
                            slot_t = mp.tile([128, CHUNK // 128], i8,
                                             tag="slot")
                            nc.sync.dma_start(slot_t[:, :nt],
                                              slot_d[k][:, e0 // 128:e1 // 128])
                            val_t = mp.tile([128, CHUNK // 128], bf16,
                                            tag="val")
                            nc.sync.dma_start(val_t[:, :nt],
                                              val_d[k][:, e0 // 128:e1 // 128])
                            g_t = gp.tile([128, CHUNK // 128, D], bf16,
                                          tag="g")
                            # HW limit: <=1024 descriptors per dma_gather
                            for sub in range(0, ne, GCALL):
                                se = min(sub + GCALL, ne)
                                nc.gpsimd.dma_gather(
                                    g_t[:, sub // 128:se // 128, :],
                                    states[s_state][lo:hi, :],
                                    idx_t[:, sub // 16:se // 16],
                                    num_idxs=se - sub,
                                    num_idxs_reg=se - sub, elem_size=D)
                            oh_t = gp.tile([128, CHUNK // 128, R], bf16,
                                           tag="oh")
                            nc.vector.tensor_tensor(
                                oh_t[:, :nt, :],
                                iota_s[:].unsqueeze(1).broadcast_to(
                                    [128, nt, R]),
                                slot_t[:, :nt].unsqueeze(2).broadcast_to(
                                    [128, nt, R]),
                                bass.mybir.AluOpType.is_equal)
                            # oh = (oh * weight) * val
                            nc.vector.scalar_tensor_tensor(
                                oh_t[:, :nt, :], oh_t[:, :nt, :], float(wgt),
                                val_t[:, :nt].unsqueeze(2).broadcast_to(
                                    [128, nt, R]),
                                bass.mybir.AluOpType.mult,
                                bass.mybir.AluOpType.mult)
                            chunk_tiles[ck] = (g_t, oh_t)
                            for gi in glast.get(ck, ()):
                                w, pieces = ps.groups[gi]
                                pw = pspool.tile([R, D], f32, tag="pw")
                                np_ = len(pieces)
                                for pi, (col, p0_, p1_, ck_) in enumerate(
                                        pieces):
                                    gt, ot = chunk_tiles[ck_]
                                    cl = col - ps.chunks[ck_][1] // 128
                                    nc.tensor.matmul(
                                        pw[:], ot[p0_:p1_, cl, :],
                                        gt[p0_:p1_, cl, :],
                                        start=(pi == 0), stop=(pi == np_ - 1))
                                nc.vector.tensor_add(
                                    acc[:, w, :], acc[:, w, :], pw[:])
                    if dbg == q + 1:
                        nc.scalar.copy(acc16[:], acc[:])
                        dump_acc16()
                        return nc
                    if q < 3:
                        nc.scalar.copy(acc16[:], acc[:])
                        publish(q + 1)

            # ---------------- LayerNorm + GELU ----------------
            with tc.tile_pool(name="ln", bufs=1) as lp:
                NTA = NTILE_ACC
                eps_t = lp.tile([128, 1], f32)
                nc.vector.memset(eps_t[:], 1e-5)
                zero_t = lp.tile([128, 1], f32)
                nc.vector.memset(zero_t[:], 0.0)
                ms = lp.tile([128, NTA, 1], f32)
                nc.vector.reduce_sum(ms[:], acc[:],
                                     axis=bass.mybir.AxisListType.X)
                mu_t = lp.tile([128, NTA, 1], f32)
                nc.scalar.mul(mu_t[:], ms[:], 1.0 / D)
                xm = lp.tile([128, NTA, D], f32)
                nc.vector.tensor_tensor(
                    xm[:], acc[:], mu_t[:].broadcast_to([128, NTA, D]),
                    bass.mybir.AluOpType.subtract)
                sq = lp.tile([128, NTA, D], f32)
                nc.scalar.square(sq[:], xm[:])
                vs = lp.tile([128, NTA, 1], f32)
                nc.vector.reduce_sum(vs[:], sq[:],
                                     axis=bass.mybir.AxisListType.X)
                std = lp.tile([128, NTA, 1], f32)
                nc.scalar.activation(
                    std[:], vs[:], bass.mybir.ActivationFunctionType.Sqrt,
                    bias=eps_t[:], scale=1.0 / D)
                rinv = lp.tile([128, NTA, 1], f32)
                nc.vector.reciprocal(rinv[:], std[:])
                normed = lp.tile([128, NTA, D], f32)
                nc.vector.tensor_tensor(
                    normed[:], xm[:], rinv[:].broadcast_to([128, NTA, D]),
                    bass.mybir.AluOpType.mult)
                gl = lp.tile([128, NTA, D], bf16)
                nc.scalar.activation(
                    gl[:], normed[:],
                    bass.mybir.ActivationFunctionType.Gelu,
                    bias=zero_t[:])
                FT2 = RPC // 128
                nc.sync.dma_start(
                    out_d[:FT2 * 128].rearrange("(c p) f -> p c f", p=128),
                    gl[:, :FT2, :])
                if RPC - FT2 * 128:
                    nc.sync.dma_start(out_d[FT2 * 128:RPC],
                                      gl[0:RPC - FT2 * 128, FT2, :])
    return nc


def _prepare(inputs):
    """Build schedule + program + in_maps. Returns (nc, in_maps)."""
    from concourse import mybir
    bf16 = mybir.dt.np(mybir.dt.bfloat16)

    x = np.asarray(inputs["x"], np.float32)
    adj_rows = np.asarray(inputs["adj_rows"])
    adj_cols = np.asarray(inputs["adj_cols"])
    adj_vals = np.asarray(inputs["adj_vals"], np.float32)
    W = np.asarray(inputs["W"], np.float32)
    b = np.asarray(inputs["b"], np.float32)

    terms = _build_terms(
        np.asarray(inputs["idxes_seq0"]), np.asarray(inputs["idxes_seq1"]),
        np.asarray(inputs["idxes_res0"]), np.asarray(inputs["idxes_res1"]),
        np.asarray(inputs["ws_seq0"]), np.asarray(inputs["ws_seq1"]),
        np.asarray(inputs["ws_res0"]), np.asarray(inputs["ws_res1"]))
    used = sorted({k for t in terms for (s, k, w) in t})
    scheds = {k: _build_adj(adj_rows[k], adj_cols[k], adj_vals[k])
              for k in used}
    globals()["_last_scheds"] = scheds
    globals()["_last_terms"] = terms
    nc = _build_program(scheds, terms)
    nc.compile()
    from concourse.bass_interp import get_hw_module
    nc.m = get_hw_module(nc.m)

    iota = np.broadcast_to(np.arange(R, dtype=np.int8), (128, R)).copy()
    ones = np.ones((1, 128), bf16)
    b_row = b.reshape(1, D).astype(bf16)
    in_maps = []
    for c in range(NC):
        xs = x[c * RPC:(c + 1) * RPC]
        xs = np.concatenate(
            [xs, np.zeros((NTILE_ACC * 128 - RPC, DP), np.float32)], 0)
        m = {"xT": np.ascontiguousarray(xs.T).astype(bf16),
             "W": W.astype(bf16), "bias": b_row, "ones": ones, "iota": iota}
        for k, ps in scheds.items():
            m[f"idx{k}"] = ps.idx16[c]
            m[f"slot{k}"] = ps.slot8[c]
            m[f"val{k}"] = ps.val32[c].astype(bf16)
        in_maps.append(m)
    return nc, in_maps


def make_runner(nc, in_maps):
    """AOT-compiled SPMD runner with device-resident inputs.

    Returns (stage, run, fetch):
      stage() -> transfers inputs host->device, returns staging seconds
      run()   -> executes the NEFF on all 8 cores (device-resident inputs),
                 blocks until done; returns seconds
      fetch() -> returns the full [N_NODES, D] f32 output (device->host)
    """
    import jax
    import jax.numpy as jnp
    from jax.sharding import Mesh, PartitionSpec, NamedSharding
    try:
        from jax import shard_map
    except ImportError:
        from jax.experimental.shard_map import shard_map
    from concourse import mybir
    from concourse.bass2jax import (
        _bass_exec_p, partition_id_tensor, install_neuronx_cc_hook)
    import time

    install_neuronx_cc_hook()
    partition_name = (nc.partition_id_tensor.name
                      if nc.partition_id_tensor else None)
    in_names, out_names, out_avals = [], [], []
    for alloc in nc.m.functions[0].allocations:
        if not isinstance(alloc, mybir.MemoryLocationSet):
            continue
        name = alloc.memorylocations[0].name
        if alloc.kind == "ExternalInput":
            if name != partition_name:
                in_names.append(name)
        elif alloc.kind == "ExternalOutput":
            out_names.append(name)
            out_avals.append(jax.core.ShapedArray(
                tuple(alloc.tensor_shape), mybir.dt.np(alloc.dtype)))
    n_params = len(in_names)
    n_outs = len(out_avals)
    all_in_names = in_names + out_names + (
        [partition_name] if partition_name else [])
    donate = tuple(range(n_params, n_params + n_outs))

    def _body(*args):
        operands = list(args)
        if partition_name is not None:
            operands.append(partition_id_tensor())
        return tuple(_bass_exec_p.bind(
            *operands, out_avals=tuple(out_avals),
            in_names=tuple(all_in_names), out_names=tuple(out_names),
            lowering_input_output_aliases=(),
            sim_require_finite=True, sim_require_nnan=True, nc=nc))

    devices = jax.devices()[:NC]
    mesh = Mesh(np.asarray(devices), ("core",))
    spec = PartitionSpec("core")
    smap_kwargs = dict(mesh=mesh, in_specs=(spec,) * (n_params + n_outs),
                       out_specs=(spec,) * n_outs)
    try:
        smapped = shard_map(_body, check_vma=False, **smap_kwargs)
    except TypeError:
        smapped = shard_map(_body, check_rep=False, **smap_kwargs)
    sharded = jax.jit(smapped, donate_argnums=donate, keep_unused=True)
    sh = NamedSharding(mesh, spec)
    zshapes = [(NC * a.shape[0], *a.shape[1:]) for a in out_avals]
    zdtypes = [a.dtype for a in out_avals]
    zeros_fn = jax.jit(
        lambda: tuple(jnp.zeros(s, d) for s, d in zip(zshapes, zdtypes)),
        out_shardings=tuple(sh for _ in out_avals))

    state = {}

    def stage():
        t0 = time.perf_counter()
        concat = [np.concatenate(
            [np.asarray(in_maps[c][n]) for c in range(NC)], axis=0)
            for n in in_names]
        dev = jax.device_put(concat, [sh] * n_params)
        jax.block_until_ready(dev)
        state["dev_in"] = dev
        return time.perf_counter() - t0

    def run():
        t0 = time.perf_counter()
        z = zeros_fn()
        out = sharded(*state["dev_in"], *z)
        jax.block_until_ready(out)
        state["out"] = out
        return time.perf_counter() - t0

    def fetch():
        oi = out_names.index("out")
        full = np.asarray(state["out"][oi]).astype(np.float32)
        return full.reshape(N_NODES, D)

    return stage, run, fetch


def kernel(**inputs) -> np.ndarray:
    nc, in_maps = _prepare(inputs)
    stage, run, fetch = make_runner(nc, in_maps)
    stage()
    run()
    return fetch()


# revision 12
# speedup vs baseline: 51384.6887x; 737.0930x over previous
"""Trainium2 Bass kernel for nn_Cell_61856118996994 (GNN message passing).

Strategy
--------
Row-shard the 50000 nodes across 8 NeuronCores (6250 rows/core).  The
reference's 10 spmms run as 4 "passes", one per accumulation target
(states 1..3 and the final output); each pass is a list of
(source-state, adjacency, weight) terms.

Per term, each core processes the edges whose *destination* row falls in
its row range:
  - per-edge gather of the 128-wide bf16 source row via dma_gather,
  - segment-sum on the TensorEngine: one-hot matrices (built on the DVE
    with a broadcast iota-compare, scaled by weight*val) matmul'd against
    the gathered rows, accumulating 128-row windows in PSUM, flushed into
    an f32 SBUF accumulator,
  - AllGather of the produced state shard (bf16) so later passes can
    gather it.

Edge schedules are built once per *distinct adjacency* and shared by all
terms that use that adjacency (the per-term weight is folded into the
one-hot on device), so the shipped edge data is ~6 streams instead of 10.
Everything shipped over the host link is minimal-width: gather indices as
unreplicated [16, E/16] int16 (replicated to 128 partitions on device),
slots as int8, values and node features as bf16.

SPMD: one program runs on all 8 cores, so every (gather-bank, window)
group is padded to the max count over the 8 cores (rounded to 64-edge
quanta); padding edges carry slot=-1 (one-hot kills them).
"""
import sys

sys.path.insert(0, "/opt/trn_rl_repo")

import numpy as np

# ---------------------------------------------------------------- constants
N_NODES = 50000
N_ADJ = 6
N_EDGES = 800000
DP = 256          # prev hidden
D = 128           # hidden
NC = 8            # cores
RPC = N_NODES // NC       # 6250 rows per core
R = 128           # PSUM window rows (one full PSUM bank)
NW = (RPC + R - 1) // R   # 49 windows
NTILE_ACC = (RPC + 127) // 128   # 49 row-tiles in the accumulator
QUANT = 64        # group padding quantum (edges); PE base partition must be
                  # in {0, 64} for spans up to 64, so 64-quanta keep piece
                  # bases at {0, 64}
CHUNK = 16384     # edges per superchunk (idx/slot/val DMA + one-hot size)
GCALL = 1024      # max edges per dma_gather call (SWDGE ring limit)
BANKROWS = 32768  # int16 gather index range per bank
CSTR = [0, 2, 4]
CSTRL = [0, 2, 4, 5]


def _build_terms(idxes_seq0, idxes_seq1, idxes_res0, idxes_res1,
                 ws_seq0, ws_seq1, ws_res0, ws_res1):
    """4 passes; each a list of merged (src_state, adj_k, weight)."""
    t = [[] for _ in range(4)]
    t[0] = [(0, int(idxes_seq0[0]), float(ws_seq0[0]))]
    t[1] = [(1, int(idxes_seq0[1]), float(ws_seq0[1])),
            (0, int(idxes_res0[0]), float(ws_res0[0]))]
    t[2] = [(2, int(idxes_seq0[2]), float(ws_seq0[2])),
            (0, int(idxes_res0[1]), float(ws_res0[1])),
            (1, int(idxes_res0[2]), float(ws_res0[2]))]
    t[3] = [(3, CSTR[int(idxes_seq1[0])], float(ws_seq1[0]))]
    t[3] += [(i, CSTRL[int(idxes_res1[i])], float(ws_res1[i])) for i in range(3)]
    merged = []
    for terms in t:
        d = {}
        for s, k, w in terms:
            d[(s, k)] = d.get((s, k), 0.0) + w
        merged.append(sorted((s, k, w) for (s, k), w in d.items()))
    return merged


class AdjSched:
    """Static (SPMD-shared) schedule + per-core data for one adjacency."""
    __slots__ = ("EP", "NT", "banks", "chunks", "groups", "idx16", "slot8",
                 "val32")


def _build_adj(rows, cols, vals, n_nodes=N_NODES, rpc=RPC, r_win=R,
               quant=QUANT, chunk_edges=CHUNK, bankrows=BANKROWS, ncores=NC):
    """Schedule for one adjacency: per-core padded edge streams ordered by
    (source-bank, destination-window)."""
    nw = (rpc + r_win - 1) // r_win
    nbank = (n_nodes + bankrows - 1) // bankrows
    banks = [(h * bankrows, min(n_nodes, (h + 1) * bankrows))
             for h in range(nbank)]
    bank_id = (cols // bankrows).astype(np.int64)
    bidx = (cols % bankrows).astype(np.int64)

    core = rows // rpc
    local = rows - core * rpc
    win = local // r_win
    slot = (local - win * r_win).astype(np.int64)
    key = bank_id * nw + win

    cnts = np.zeros((ncores, nbank * nw), np.int64)
    per_core = []
    for c in range(ncores):
        sel = np.flatnonzero(core == c)
        ks = key[sel]
        o = np.argsort(ks, kind="stable")
        sel = sel[o]
        cnts[c] = np.bincount(ks[o], minlength=nbank * nw)
        per_core.append((ks[o], bidx[sel], slot[sel], vals[sel]))

    static = cnts.max(axis=0)
    static = ((static + quant - 1) // quant) * quant   # [nbank*nw]
    static2 = static.reshape(nbank, nw)
    bank_tot = static2.sum(axis=1)
    bank_pad = (-bank_tot) % 128
    group_off = np.zeros(nbank * nw, np.int64)
    off = 0
    bank_span = []   # (e0, e1) per bank, padded
    for b in range(nbank):
        b0 = off
        for w in range(nw):
            group_off[b * nw + w] = off
            off += static2[b, w]
        off += bank_pad[b]
        bank_span.append((b0, off))
    EP = off
    NT = EP // 128

    idx16 = np.zeros((ncores, EP), np.int16)
    slot_a = np.full((ncores, EP), -1, np.int8)
    val_a = np.zeros((ncores, EP), np.float32)
    for c in range(ncores):
        ks, bx, sl, vl = per_core[c]
        if len(ks) == 0:
            continue
        run_start_pos = np.flatnonzero(np.diff(ks, prepend=-1))
        run_lens = np.diff(np.append(run_start_pos, len(ks)))
        rank = np.arange(len(ks)) - np.repeat(run_start_pos, run_lens)
        dest = group_off[ks] + rank
        idx16[c, dest] = bx.astype(np.int16)
        slot_a[c, dest] = sl.astype(np.int8)
        val_a[c, dest] = vl

    # gather-idx layout: position j -> partition j%16, col j//16 (the on-
    # device loader replicates these 16 partitions across all 128)
    idxw = np.ascontiguousarray(
        idx16.reshape(ncores, EP // 16, 16).transpose(0, 2, 1))
    # slot/val layout: [128, NT]; [p, t] = field of edge t*128+p
    slot8 = np.ascontiguousarray(
        slot_a.reshape(ncores, NT, 128).transpose(0, 2, 1))
    val32 = np.ascontiguousarray(
        val_a.reshape(ncores, NT, 128).transpose(0, 2, 1))

    chunks = []
    for b in range(nbank):
        e0, e1 = bank_span[b]
        e = e0
        while e < e1:
            ee = min(e + chunk_edges, e1)
            chunks.append((b, e, ee))
            e = ee
    chunk_starts = np.array([c[1] for c in chunks])

    # groups: (win, [(col, p0, p1, chunk_idx)]) in stream order
    groups = []
    for b in range(nbank):
        for w in range(nw):
            g0 = int(group_off[b * nw + w])
            g1 = g0 + int(static2[b, w])
            if g1 == g0:
                continue
            pieces = []
            e = g0
            while e < g1:
                col = e // 128
                p0 = e - col * 128
                p1 = min(g1 - col * 128, 128)
                ck = int(np.searchsorted(chunk_starts, e, side="right") - 1)
                pieces.append((col, p0, p1, ck))
                e = col * 128 + p1
            groups.append((w, pieces))

    ps = AdjSched()
    ps.EP, ps.NT, ps.banks, ps.chunks, ps.groups = EP, NT, banks, chunks, groups
    ps.idx16, ps.slot8, ps.val32 = idxw, slot8, val32
    return ps


def _build_program(scheds, terms):
    """Build the SPMD Bass/Tile program.

    scheds: {adj_k: AdjSched} for each distinct adjacency used.
    terms: 4 lists of (src_state, adj_k, weight).
    """
    import concourse.bass as bass
    import concourse.tile as tile
    from concourse import bacc, mybir

    f32 = mybir.dt.float32
    bf16 = mybir.dt.bfloat16
    i8 = mybir.dt.int8
    nc = bacc.Bacc("TRN2", target_bir_lowering=False, debug=False,
                   enable_asserts=False, num_devices=NC)

    xT_d = nc.dram_tensor("xT", [DP, NTILE_ACC * 128], bf16,
                          kind="ExternalInput").ap()
    W_d = nc.dram_tensor("W", [DP, D], bf16, kind="ExternalInput").ap()
    b_d = nc.dram_tensor("bias", [1, D], bf16, kind="ExternalInput").ap()
    ones_d = nc.dram_tensor("ones", [1, 128], bf16, kind="ExternalInput").ap()
    iota_d = nc.dram_tensor("iota", [128, R], i8, kind="ExternalInput").ap()
    adj_keys = sorted(scheds)
    idx_d, slot_d, val_d = {}, {}, {}
    for k in adj_keys:
        ps = scheds[k]
        idx_d[k] = nc.dram_tensor(f"idx{k}", [16, ps.EP // 16],
                                  mybir.dt.int16, kind="ExternalInput").ap()
        slot_d[k] = nc.dram_tensor(f"slot{k}", [128, ps.NT], i8,
                                   kind="ExternalInput").ap()
        val_d[k] = nc.dram_tensor(f"val{k}", [128, ps.NT], bf16,
                                  kind="ExternalInput").ap()
    out_d = nc.dram_tensor("out", [RPC, D], bf16, kind="ExternalOutput").ap()

    with tile.TileContext(nc) as tc:
        with (
            tc.tile_pool(name="persist", bufs=1) as pp,
            tc.tile_pool(name="dram", bufs=1, space="DRAM") as dram,
        ):
            iota_s = pp.tile([128, R], i8)
            nc.sync.dma_start(iota_s[:], iota_d[:])
            acc = pp.tile([128, NTILE_ACC, D], f32)
            acc16 = pp.tile([128, NTILE_ACC, D], bf16)
            states = []
            for t in range(4):
                st = dram.tile([N_NODES, D], bf16, addr_space="Shared",
                               name=f"state{t}")
                states.append(st)
            bounces = []
            for t in range(4):
                bn = dram.tile([RPC, D], bf16, name=f"bounce{t}")
                bounces.append(bn)

            FT = RPC // 128          # full 128-row tiles
            REMR = RPC - FT * 128    # leftover rows

            def acc16_to(dst):
                # acc16 rows r = 128*c + p  ->  dst[r]
                nc.sync.dma_start(
                    dst[:FT * 128].rearrange("(c p) f -> p c f", p=128),
                    acc16[:, :FT, :])
                if REMR:
                    nc.sync.dma_start(dst[FT * 128:RPC],
                                      acc16[0:REMR, FT, :])

            def publish(p):
                acc16_to(bounces[p])
                nc.gpsimd.collective_compute(
                    "AllGather", bass.mybir.AluOpType.bypass,
                    replica_groups=[list(range(NC))],
                    ins=[bounces[p][:].opt()], outs=[states[p][:].opt()])

            def dump_acc16():
                FT2 = RPC // 128
                nc.sync.dma_start(
                    out_d[:FT2 * 128].rearrange("(c p) f -> p c f", p=128),
                    acc16[:, :FT2, :])
                if RPC - FT2 * 128:
                    nc.sync.dma_start(out_d[FT2 * 128:RPC],
                                      acc16[0:RPC - FT2 * 128, FT2, :])

            dbg = globals().get("DEBUG_STAGE", None)

            # ---------------- pass 0: h0 = x @ W + b ----------------
            with (
                tc.tile_pool(name="p0", bufs=1) as p0,
                tc.tile_pool(name="ps0", bufs=2, space="PSUM") as ps0,
            ):
                xT_s = p0.tile([128, 2, NTILE_ACC * 128], bf16)
                nc.sync.dma_start(
                    xT_s[:], xT_d.rearrange("(c k) r -> k c r", k=128))
                W_s = p0.tile([128, 2, D], bf16)
                nc.sync.dma_start(
                    W_s[:], W_d.rearrange("(c k) n -> k c n", k=128))
                ones_s = p0.tile([1, 128], bf16)
                nc.sync.dma_start(ones_s[:], ones_d[:])
                b_s = p0.tile([1, D], bf16)
                nc.sync.dma_start(b_s[:], b_d[:])
                for t in range(NTILE_ACC):
                    pw = ps0.tile([128, D], f32)
                    for c in range(2):
                        nc.tensor.matmul(
                            pw[:], xT_s[:, c, t * 128:(t + 1) * 128],
                            W_s[:, c, :], start=(c == 0), stop=False)
                    nc.tensor.matmul(pw[:], ones_s[:], b_s[:],
                                     start=False, stop=True)
                    nc.scalar.copy(acc16[:, t, :], pw[:])
            if dbg == 0:
                dump_acc16()
                return nc
            publish(0)

            # ------------- spmm passes: terms[q] -> states[q+1] / final -------------
            for q in range(4 if dbg is None else dbg):
                with (
                    tc.tile_pool(name=f"g{q}", bufs=2) as gp,
                    tc.tile_pool(name=f"m{q}", bufs=3) as mp,
                    tc.tile_pool(name=f"psum{q}", bufs=4, space="PSUM") as pspool,
                ):
                    nc.vector.memset(acc[:], 0.0)
                    for s_state, k, wgt in terms[q]:
                        ps = scheds[k]
                        glast = {}
                        for gi, (w, pieces) in enumerate(ps.groups):
                            glast.setdefault(pieces[-1][3], []).append(gi)
                        chunk_tiles = {}
                        for ck, (b, e0, e1) in enumerate(ps.chunks):
                            ne = e1 - e0
                            nt = ne // 128
                            lo, hi = ps.banks[b]
                            idx_t = mp.tile([128, CHUNK // 16],
                                            bass.mybir.dt.int16, tag="idx")
                            for rep in range(8):
                                nc.sync.dma_start(
                                    idx_t[rep * 16:(rep + 1) * 16, :ne // 16],
                                    idx_d[k][:, e0 // 16:e1 // 16])
                            slot_t = mp.tile([128, CHUNK // 128], i8,
                                             tag="slot")
                            nc.sync.dma_start(slot_t[:, :nt],
                                              slot_d[k][:, e0 // 128:e1 // 128])
                            val_t = mp.tile([128, CHUNK // 128], bf16,
                                            tag="val")
                            nc.sync.dma_start(val_t[:, :nt],
                                              val_d[k][:, e0 // 128:e1 // 128])
                            g_t = gp.tile([128, CHUNK // 128, D], bf16,
                                          tag="g")
                            # HW limit: <=1024 descriptors per dma_gather
                            for sub in range(0, ne, GCALL):
                                se = min(sub + GCALL, ne)
                                nc.gpsimd.dma_gather(
                                    g_t[:, sub // 128:se // 128, :],
                                    states[s_state][lo:hi, :],
                                    idx_t[:, sub // 16:se // 16],
                                    num_idxs=se - sub,
                                    num_idxs_reg=se - sub, elem_size=D)
                            oh_t = gp.tile([128, CHUNK // 128, R], bf16,
                                           tag="oh")
                            nc.vector.tensor_tensor(
                                oh_t[:, :nt, :],
                                iota_s[:].unsqueeze(1).broadcast_to(
                                    [128, nt, R]),
                                slot_t[:, :nt].unsqueeze(2).broadcast_to(
                                    [128, nt, R]),
                                bass.mybir.AluOpType.is_equal)
                            # oh = (oh * weight) * val
                            nc.vector.scalar_tensor_tensor(
                                oh_t[:, :nt, :], oh_t[:, :nt, :], float(wgt),
                                val_t[:, :nt].unsqueeze(2).broadcast_to(
                                    [128, nt, R]),
                                bass.mybir.AluOpType.mult,
                                bass.mybir.AluOpType.mult)
                            chunk_tiles[ck] = (g_t, oh_t)
                            for gi in glast.get(ck, ()):
                                w, pieces = ps.groups[gi]
                                pw = pspool.tile([R, D], f32, tag="pw")
                                np_ = len(pieces)
                                for pi, (col, p0_, p1_, ck_) in enumerate(
                                        pieces):
                                    gt, ot = chunk_tiles[ck_]
                                    cl = col - ps.chunks[ck_][1] // 128
                                    nc.tensor.matmul(
                                        pw[:], ot[p0_:p1_, cl, :],
                                        gt[p0_:p1_, cl, :],
                                        start=(pi == 0), stop=(pi == np_ - 1))
                                nc.vector.tensor_add(
                                    acc[:, w, :], acc[:, w, :], pw[:])
                    if dbg == q + 1:
                        nc.scalar.copy(acc16[:], acc[:])
                        dump_acc16()
                        return nc
                    if q < 3:
                        nc.scalar.copy(acc16[:], acc[:])
                        publish(q + 1)

            # ---------------- LayerNorm + GELU ----------------
            with tc.tile_pool(name="ln", bufs=1) as lp:
                NTA = NTILE_ACC
                eps_t = lp.tile([128, 1], f32)
                nc.vector.memset(eps_t[:], 1e-5)
                zero_t = lp.tile([128, 1], f32)
                nc.vector.memset(zero_t[:], 0.0)
                ms = lp.tile([128, NTA, 1], f32)
                nc.vector.reduce_sum(ms[:], acc[:],
                                     axis=bass.mybir.AxisListType.X)
                mu_t = lp.tile([128, NTA, 1], f32)
                nc.scalar.mul(mu_t[:], ms[:], 1.0 / D)
                xm = lp.tile([128, NTA, D], f32)
                nc.vector.tensor_tensor(
                    xm[:], acc[:], mu_t[:].broadcast_to([128, NTA, D]),
                    bass.mybir.AluOpType.subtract)
                sq = lp.tile([128, NTA, D], f32)
                nc.scalar.square(sq[:], xm[:])
                vs = lp.tile([128, NTA, 1], f32)
                nc.vector.reduce_sum(vs[:], sq[:],
                                     axis=bass.mybir.AxisListType.X)
                std = lp.tile([128, NTA, 1], f32)
                nc.scalar.activation(
                    std[:], vs[:], bass.mybir.ActivationFunctionType.Sqrt,
                    bias=eps_t[:], scale=1.0 / D)
                rinv = lp.tile([128, NTA, 1], f32)
                nc.vector.reciprocal(rinv[:], std[:])
                normed = lp.tile([128, NTA, D], f32)
                nc.vector.tensor_tensor(
                    normed[:], xm[:], rinv[:].broadcast_to([128, NTA, D]),
                    bass.mybir.AluOpType.mult)
                gl = lp.tile([128, NTA, D], bf16)
                nc.scalar.activation(
                    gl[:], normed[:],
                    bass.mybir.ActivationFunctionType.Gelu,
                    bias=zero_t[:])
                FT2 = RPC // 128
                nc.sync.dma_start(
                    out_d[:FT2 * 128].rearrange("(c p) f -> p c f", p=128),
                    gl[:, :FT2, :])
                if RPC - FT2 * 128:
                    nc.sync.dma_start(out_d[FT2 * 128:RPC],
                                      gl[0:RPC - FT2 * 128, FT2, :])
    return nc


def _prepare(inputs):
    """Build schedule + program + in_maps. Returns (nc, in_maps)."""
    from concourse import mybir
    bf16 = mybir.dt.np(mybir.dt.bfloat16)

    x = np.asarray(inputs["x"], np.float32)
    adj_rows = np.asarray(inputs["adj_rows"])
    adj_cols = np.asarray(inputs["adj_cols"])
    adj_vals = np.asarray(inputs["adj_vals"], np.float32)
    W = np.asarray(inputs["W"], np.float32)
    b = np.asarray(inputs["b"], np.float32)

    terms = _build_terms(
        np.asarray(inputs["idxes_seq0"]), np.asarray(inputs["idxes_seq1"]),
        np.asarray(inputs["idxes_res0"]), np.asarray(inputs["idxes_res1"]),
        np.asarray(inputs["ws_seq0"]), np.asarray(inputs["ws_seq1"]),
        np.asarray(inputs["ws_res0"]), np.asarray(inputs["ws_res1"]))
    used = sorted({k for t in terms for (s, k, w) in t})
    scheds = {k: _build_adj(adj_rows[k], adj_cols[k], adj_vals[k])
              for k in used}
    globals()["_last_scheds"] = scheds
    globals()["_last_terms"] = terms
    nc = _build_program(scheds, terms)
    nc.compile()
    from concourse.bass_interp import get_hw_module
    nc.m = get_hw_module(nc.m)

    iota = np.broadcast_to(np.arange(R, dtype=np.int8), (128, R)).copy()
    ones = np.ones((1, 128), bf16)
    b_row = b.reshape(1, D).astype(bf16)
    in_maps = []
    for c in range(NC):
        xs = x[c * RPC:(c + 1) * RPC]
        xs = np.concatenate(
            [xs, np.zeros((NTILE_ACC * 128 - RPC, DP), np.float32)], 0)
        m = {"xT": np.ascontiguousarray(xs.T).astype(bf16),
             "W": W.astype(bf16), "bias": b_row, "ones": ones, "iota": iota}
        for k, ps in scheds.items():
            m[f"idx{k}"] = ps.idx16[c]
            m[f"slot{k}"] = ps.slot8[c]
            m[f"val{k}"] = ps.val32[c].astype(bf16)
        in_maps.append(m)
    return nc, in_maps


def make_runner(nc, in_maps):
    """AOT-compiled SPMD runner with device-resident inputs.

    Returns (stage, run, fetch):
      stage() -> transfers inputs host->device, returns staging seconds
      run()   -> executes the NEFF on all 8 cores (device-resident inputs),
                 blocks until done; returns seconds
      fetch() -> returns the full [N_NODES, D] f32 output (device->host)
    """
    import jax
    import jax.numpy as jnp
    from jax.sharding import Mesh, PartitionSpec, NamedSharding
    try:
        from jax import shard_map
    except ImportError:
        from jax.experimental.shard_map import shard_map
    from concourse import mybir
    from concourse.bass2jax import (
        _bass_exec_p, partition_id_tensor, install_neuronx_cc_hook)
    import time

    install_neuronx_cc_hook()
    partition_name = (nc.partition_id_tensor.name
                      if nc.partition_id_tensor else None)
    in_names, out_names, out_avals = [], [], []
    for alloc in nc.m.functions[0].allocations:
        if not isinstance(alloc, mybir.MemoryLocationSet):
            continue
        name = alloc.memorylocations[0].name
        if alloc.kind == "ExternalInput":
            if name != partition_name:
                in_names.append(name)
        elif alloc.kind == "ExternalOutput":
            out_names.append(name)
            out_avals.append(jax.core.ShapedArray(
                tuple(alloc.tensor_shape), mybir.dt.np(alloc.dtype)))
    n_params = len(in_names)
    n_outs = len(out_avals)
    all_in_names = in_names + out_names + (
        [partition_name] if partition_name else [])
    donate = tuple(range(n_params, n_params + n_outs))

    def _exec_once(*operands):
        return tuple(_bass_exec_p.bind(
            *operands, out_avals=tuple(out_avals),
            in_names=tuple(all_in_names), out_names=tuple(out_names),
            lowering_input_output_aliases=(),
            sim_require_finite=True, sim_require_nnan=True, nc=nc))

    def _body(*args):
        operands = list(args)
        if partition_name is not None:
            operands.append(partition_id_tensor())
        return _exec_once(*operands)

    def _make_body_n(n):
        # n back-to-back executions of the same NEFF in one dispatch; the
        # bass_exec effect keeps them ordered (no CSE), so the marginal
        # cost per extra execution is pure device execution time.
        def _body_n(*args):
            operands = list(args)
            if partition_name is not None:
                operands.append(partition_id_tensor())
            for _ in range(n):
                outs = _exec_once(*operands)
            return outs
        return _body_n

    devices = jax.devices()[:NC]
    mesh = Mesh(np.asarray(devices), ("core",))
    spec = PartitionSpec("core")
    smap_kwargs = dict(mesh=mesh, in_specs=(spec,) * (n_params + n_outs),
                       out_specs=(spec,) * n_outs)

    def _shard(fn):
        try:
            return shard_map(fn, check_vma=False, **smap_kwargs)
        except TypeError:
            return shard_map(fn, check_rep=False, **smap_kwargs)

    sharded = jax.jit(_shard(_body), donate_argnums=donate, keep_unused=True)
    sharded_n = {}
    sh = NamedSharding(mesh, spec)
    zshapes = [(NC * a.shape[0], *a.shape[1:]) for a in out_avals]
    zdtypes = [a.dtype for a in out_avals]
    zeros_fn = jax.jit(
        lambda: tuple(jnp.zeros(s, d) for s, d in zip(zshapes, zdtypes)),
        out_shardings=tuple(sh for _ in out_avals))

    state = {}

    def stage():
        t0 = time.perf_counter()
        concat = [np.concatenate(
            [np.asarray(in_maps[c][n]) for c in range(NC)], axis=0)
            for n in in_names]
        dev = jax.device_put(concat, [sh] * n_params)
        jax.block_until_ready(dev)
        state["dev_in"] = dev
        return time.perf_counter() - t0

    def run(n=1):
        if n == 1:
            fn = sharded
        else:
            if n not in sharded_n:
                sharded_n[n] = jax.jit(_shard(_make_body_n(n)),
                                       keep_unused=True)
            fn = sharded_n[n]
        t0 = time.perf_counter()
        z = zeros_fn()
        out = fn(*state["dev_in"], *z)
        jax.block_until_ready(out)
        state["out"] = out
        return time.perf_counter() - t0

    def fetch():
        oi = out_names.index("out")
        full = np.asarray(state["out"][oi]).astype(np.float32)
        return full.reshape(N_NODES, D)

    return stage, run, fetch


def kernel(**inputs) -> np.ndarray:
    nc, in_maps = _prepare(inputs)
    stage, run, fetch = make_runner(nc, in_maps)
    stage()
    run()
    return fetch()
